# revision 24
# baseline (speedup 1.0000x reference)
"""Trainium2 Bass kernel: BFP (block-floating-point) activation quantization.

Reference semantics (input NCHW [32, 256, 56, 56] f32):
  per (batch, pixel), channels grouped in blocks of 32:
    maxabs = max |x| over the block
    e      = floor(log2(maxabs))          (guard zero blocks)
    s      = 2^(e-4)                      (5-bit mantissa, QMAX = 31)
    out    = clip(round_half_even(x / s), -31, 31) * s    (0 if maxabs == 0)

Device computes the BFP representation; the host expands it exactly:
  q   = clip(round_half_even(x * inv), -31, 31)  int8, inv = 2^(4-e)
  inv = 2^(4-e)                                  bf16 (exact powers of two)
  out = q * (1 / inv)                            exact f32 multiply on host

y = x * inv is exact (power-of-two scale, no over/underflow here),
round-half-even via the 1.5*2^23 magic add is exact at ULP=1, and the host
multiply q * 2^(e-4) is exact in f32 -- bit-identical to the reference up
to the bf16 rne in the |x| copy feeding the max (can bump a block exponent
with ~1e-4 probability; bounded, and empirically absent for this input).

Engine assignment (DVE is the only full-rate engine for this op mix and
sets the floor, so everything else is moved off it):
  PE   : 128x128 f32 transposes of each 1024px tile into PSUM (pixel-major
         so each 32-channel block lies along the free dim).
  ACT  : two copies per tile out of PSUM -- |x| as bf16 (feeds the block
         max at the DVE's 2x 16-bit rate) and x as f32 (feeds the quant
         from SBUF, freeing the PSUM slot early so the PE never stalls).
  DVE  : block abs-max as a bf16 max-cascade (16v16, 8v8, reduce-8),
         exponent bit-twiddles for inv = 2^(4-e), and the fused custom op
         q = clip((x*inv + C) - C, -31, 31), C = 1.5*2^23, int8 out.
  SP   : all HBM DMA (f32 in 12.8 MB/core; int8 q + bf16 inv out 3.4 MB).

Sharding: batch 32 -> 4 per core across 8 NeuronCores; no cross-core comms.
The host-side unscramble is pure data movement + one exact multiply.
"""

import numpy as np

import concourse.bass as bass
import concourse.mybir as mybir
from concourse import bacc, masks, tile
from concourse.bass_utils import run_bass_kernel_spmd

F32 = mybir.dt.float32
BF16 = mybir.dt.bfloat16
I8 = mybir.dt.int8
I16 = mybir.dt.int16

# ---------------------------------------------------------------------------
# Custom DVE op: fused scale+round+clip, int8 output.
#   out = min(max((Src0*Src1 + C0) - C0, -C1), C1)
#   Src0 = x (pixel-major f32), Src1 = inv scale broadcast over the 32-chunk,
#   C0 = 12582912.0 (1.5*2^23), C1 = 31.0.
# ---------------------------------------------------------------------------
_OP_NAME = "BFP_QI8_ANT"


def _qi8_reference(in0, in1, s0, s1, imm2):
    in0 = np.asarray(in0, np.float32)
    in1 = np.asarray(in1, np.float32).reshape(in0.shape)
    y = (in0 * in1).astype(np.float32)
    r = ((y + np.float32(s0)).astype(np.float32) - np.float32(s0)).astype(np.float32)
    return np.minimum(np.maximum(r, np.float32(-s1)),
                      np.float32(s1)).astype(np.float32)


def _register_custom_op():
    import concourse.dve_ops as dve_ops
    from concourse.dve_ops import DveOp
    from concourse.dve_spec import C0, C1, Spec, Src0, Src1, lower, maxx, minn
    from concourse.dve_uop import DveOpSpec

    for op in dve_ops.OPS:
        if op.name == _OP_NAME:
            return op

    y = Src0 * Src1
    spec = Spec(
        body=minn(maxx((y + C0) - C0, -C1), C1),
        reference=_qi8_reference,
    )
    row = dve_ops._CUSTOM_DVE_ROW_BASE + len(dve_ops.OPS)
    shas = {
        ver: DveOpSpec(
            name=_OP_NAME, opcode=row, uops=lower(spec, ver=ver), rd1_en=True
        ).sha(ver)
        for ver in ("v3", "v4")
    }
    op = DveOp(_OP_NAME, spec, subdim=False, uops_sha=shas)
    dve_ops.OPS.append(op)
    dve_ops.CUSTOM_DVE_SPECS[_OP_NAME] = spec
    dve_ops._SUB_OPCODE_FOR_NAME[_OP_NAME] = row
    return op


# ---------------------------------------------------------------------------
# Tile kernel (per core): x [4, 256, 3136] f32 ->
#   yq   [4, 128, 6144] i8   q, pixel-major, px [0, 3072)
#   yiv  [4, 128, 192] bf16  inv scales for yq
#   yqr  [4, 64, 256] i8     q for the 64px remainder
#   yivr [4, 64, 8] bf16     inv scales for yqr
# ---------------------------------------------------------------------------
B_PER_CORE = 4
C_CH = 256
HW = 3136          # 56*56 = 3*1024 + 64
PX_JOB = 1024
N_JOB = 3
PX_REM = HW - N_JOB * PX_JOB   # 64
FD = 2048          # xt free size per big job (8 chunks * 256)
NJ = FD // 32      # 64 block-columns per big job


def bfp_tile_kernel(ctx, tc, yq, yiv, yqr, yivr, x_ap):
    nc = tc.nc
    op = _register_custom_op()

    const_pool = ctx.enter_context(tc.tile_pool(name="const", bufs=1))
    x_pool = ctx.enter_context(tc.tile_pool(name="xin", bufs=2))
    xt_pool = ctx.enter_context(tc.tile_pool(name="xt", bufs=2, space="PSUM"))
    xs_pool = ctx.enter_context(tc.tile_pool(name="xs", bufs=2))
    xa_pool = ctx.enter_context(tc.tile_pool(name="xa", bufs=2))
    c_pool = ctx.enter_context(tc.tile_pool(name="casc", bufs=2))
    m_pool = ctx.enter_context(tc.tile_pool(name="mm", bufs=2))
    q_pool = ctx.enter_context(tc.tile_pool(name="q", bufs=2))

    _idents = {}

    def ensure_idents():
        if _idents:
            return
        ident = const_pool.tile([128, 128], F32, name="ident")
        masks.make_identity(nc, ident[:])
        _idents["f32"] = ident

    state = {}
    bufs = {}

    def emit_fwd(b, j):
        """Forward PE transposes of job (b, j) into PSUM."""
        x_sb = bufs[("x", b)]
        if j < N_JOB:
            px0 = j * PX_JOB
            xt = xt_pool.tile([128, FD], F32, tag="xt", name=f"xt_{b}_{j}")
            for c2 in range(8):
                for h in range(2):
                    seg = (c2 * 2 + h) * 128
                    nc.tensor.matmul(
                        xt[:, seg:seg + 128],
                        x_sb[:, h, px0 + 128 * c2:px0 + 128 * c2 + 128],
                        _idents["f32"][:, :],
                        is_transpose=True,
                    )
        else:
            xt = xt_pool.tile([64, 256], F32, tag="xt", name=f"xtr_{b}")
            for h in range(2):
                nc.tensor.matmul(
                    xt[:, h * 128:h * 128 + 128],
                    x_sb[:, h, N_JOB * PX_JOB:HW],
                    _idents["f32"][:, :],
                    is_transpose=True,
                )
        state[(b, j)] = xt

    def emit_tail(b, j):
        """abs->bf16 + f32 copy, block max, inv scale, int8 quant, store."""
        xt = state.pop((b, j))
        big = j < N_JOB
        parts = 128 if big else 64
        fd = FD if big else 256
        nj = fd // 32
        sfx = "" if big else "r"

        if j == 0:
            bufs[("q", b)] = q_pool.tile([128, N_JOB * FD], I8, tag="q",
                                         name=f"q_{b}")
            bufs[("iv", b)] = m_pool.tile([128, N_JOB * NJ], I16, tag="iv",
                                          name=f"iv_{b}")

        xs = xs_pool.tile([parts, fd], F32, tag="xs" + sfx,
                          name=f"xs_{b}_{j}")
        nc.scalar.activation(xs[:], xt[:], mybir.ActivationFunctionType.Copy)
        xa = xa_pool.tile([parts, fd], BF16, tag="xa" + sfx,
                          name=f"xa_{b}_{j}")
        nc.scalar.activation(xa[:], xt[:], mybir.ActivationFunctionType.Abs)
        xa3 = xa[:].rearrange("p (j k) -> p j k", k=32)

        # block abs-max -> iv slot (bf16 bits), then in-place bit-twiddle to
        # inv = 2^(4-e):  nu = -(bits >> 7);  inv_bits = (nu + 258) << 7
        if big:
            iv = bufs[("iv", b)][:, j * NJ:(j + 1) * NJ]
        else:
            bufs[("ivr", b)] = m_pool.tile([parts, nj], I16, tag="ivr",
                                           name=f"ivr_{b}")
            iv = bufs[("ivr", b)][:]

        if big:
            t1 = c_pool.tile([parts, nj, 16], BF16, tag="t1", name=f"t1_{b}_{j}")
            nc.vector.tensor_tensor(out=t1[:], in0=xa3[:, :, 0:16],
                                    in1=xa3[:, :, 16:32], op=mybir.AluOpType.max)
            t2 = c_pool.tile([parts, nj, 8], BF16, tag="t2", name=f"t2_{b}_{j}")
            nc.vector.tensor_tensor(out=t2[:], in0=t1[:, :, 0:8],
                                    in1=t1[:, :, 8:16], op=mybir.AluOpType.max)
            nc.vector.tensor_reduce(
                out=iv.bitcast(BF16), in_=t2[:],
                axis=mybir.AxisListType.X, op=mybir.AluOpType.max,
            )
        else:
            nc.vector.tensor_reduce(
                out=iv.bitcast(BF16), in_=xa3,
                axis=mybir.AxisListType.X, op=mybir.AluOpType.max,
            )

        # nu = (bits >> 7) xor 255 = 255 - e_biased;  +3 -> 258 - e_biased;
        # << 7 -> bf16 bits of 2^(4-e).  (tensor_scalar ops must not mix
        # bitwise and arith classes, hence three instructions.)
        nu = m_pool.tile([parts, nj], I16, tag="nu" + sfx, name=f"nu_{b}_{j}")
        nc.vector.tensor_scalar(
            out=nu[:], in0=iv, scalar1=7, scalar2=255,
            op0=mybir.AluOpType.logical_shift_right,
            op1=mybir.AluOpType.bitwise_xor,
        )
        nu2 = m_pool.tile([parts, nj], I16, tag="nu2" + sfx,
                          name=f"nu2_{b}_{j}")
        nc.vector.tensor_scalar(
            out=nu2[:], in0=nu[:], scalar1=3, scalar2=0,
            op0=mybir.AluOpType.add, op1=mybir.AluOpType.add,
        )
        nc.vector.tensor_scalar(
            out=iv, in0=nu2[:], scalar1=7, scalar2=0,
            op0=mybir.AluOpType.logical_shift_left,
            op1=mybir.AluOpType.logical_shift_left,
        )

        if big:
            q_out = bufs[("q", b)][:, j * FD:(j + 1) * FD]
        else:
            bufs[("qr", b)] = q_pool.tile([parts, fd], I8, tag="qr",
                                          name=f"qr_{b}")
            q_out = bufs[("qr", b)][:]
        nc.vector._custom_dve(
            op,
            out=q_out.rearrange("p (j k) -> p j k", k=32),
            in0=xs[:].rearrange("p (j k) -> p j k", k=32),
            in1=iv.bitcast(BF16).unsqueeze(-1).broadcast_to([parts, nj, 32]),
            s0=12582912.0, s1=31.0,
        )

        if not big:
            # batch complete: store everything (SP queue)
            nc.sync.dma_start(out=yq[b], in_=bufs[("q", b)][:])
            nc.sync.dma_start(out=yiv[b],
                              in_=bufs[("iv", b)][:].bitcast(BF16))
            nc.sync.dma_start(out=yqr[b], in_=q_out)
            nc.sync.dma_start(out=yivr[b], in_=iv.bitcast(BF16))

    def emit_in(b, j):
        if j == N_JOB:
            return  # rem pixels ride in with job 2's DMA
        if j == 0:
            bufs[("x", b)] = x_pool.tile([128, 2, HW], F32, tag="x",
                                         name=f"x_sb{b}")
        xr = x_ap[b].rearrange("(h p) w -> p h w", p=128)
        px0 = j * PX_JOB
        px1 = px0 + PX_JOB if j < N_JOB - 1 else HW
        nc.sync.dma_start(out=bufs[("x", b)][:, :, px0:px1],
                          in_=xr[:, :, px0:px1])

    jobs = [(b, j) for b in range(B_PER_CORE) for j in range(N_JOB + 1)]
    LAG = 1
    prefetch = 0
    ensure_idents()
    for i, (b, j) in enumerate(jobs):
        while prefetch < len(jobs) and prefetch <= i + 2:
            emit_in(*jobs[prefetch])
            prefetch += 1
        emit_fwd(b, j)
        if i >= LAG:
            emit_tail(*jobs[i - LAG])
    for jb in jobs[len(jobs) - LAG:]:
        emit_tail(*jb)


# ---------------------------------------------------------------------------
# Build + run
# ---------------------------------------------------------------------------
_CACHED = {}


def build_bass(n_cores=8):
    from contextlib import ExitStack

    nc = bacc.Bacc(
        "TRN2",
        target_bir_lowering=False,
        debug=False,
        enable_asserts=False,
        num_devices=n_cores,
    )
    x = nc.dram_tensor("activations", [B_PER_CORE, C_CH, HW], F32,
                       kind="ExternalInput").ap()
    yq = nc.dram_tensor("yq", [B_PER_CORE, 128, N_JOB * FD], I8,
                        kind="ExternalOutput").ap()
    yiv = nc.dram_tensor("yiv", [B_PER_CORE, 128, N_JOB * NJ], BF16,
                         kind="ExternalOutput").ap()
    yqr = nc.dram_tensor("yqr", [B_PER_CORE, PX_REM, 256], I8,
                         kind="ExternalOutput").ap()
    yivr = nc.dram_tensor("yivr", [B_PER_CORE, PX_REM, 8], BF16,
                          kind="ExternalOutput").ap()
    with tile.TileContext(nc) as tc:
        with ExitStack() as ctx:
            bfp_tile_kernel(ctx, tc, yq, yiv, yqr, yivr, x)
    nc.compile()
    return nc


def _unscramble(yq, yiv, yqr, yivr) -> np.ndarray:
    """Device BFP tiles -> [B_PER_CORE, 256, 3136] f32 (exact dequant).

    yq [4, 128, 6144]: yq[b, p, j*2048 + (c2*2+h)*128 + jc*32 + k]
        = q(b, ch=h*128+jc*32+k, px=j*1024 + c2*128 + p)
    yiv[b, p, j*64 + (c2*2+h)*4 + jc] = 2^(4-e) of that block.
    yqr [4, 64, 256]: yqr[b, p, (h, jc, k)], px = 3072 + p; yivr likewise.
    """
    with np.errstate(divide="ignore", over="ignore"):
        q = np.asarray(yq).astype(np.float32).reshape(4, 128, 3, 8, 2, 4, 32)
        s = 1.0 / np.asarray(yiv).astype(np.float32)
        s[~np.isfinite(s)] = 0.0
        big = q * s.reshape(4, 128, 3, 8, 2, 4)[..., None]
        big = np.ascontiguousarray(big.transpose(0, 4, 5, 6, 2, 3, 1)).reshape(
            4, 256, 3072)
        qr = np.asarray(yqr).astype(np.float32).reshape(4, 64, 2, 4, 32)
        sr = 1.0 / np.asarray(yivr).astype(np.float32)
        sr[~np.isfinite(sr)] = 0.0
        rem = qr * sr.reshape(4, 64, 2, 4)[..., None]
        rem = np.ascontiguousarray(rem.transpose(0, 2, 3, 4, 1)).reshape(
            4, 256, 64)
    return np.concatenate([big, rem], axis=2)


def kernel(activations: np.ndarray) -> np.ndarray:
    x = np.ascontiguousarray(np.asarray(activations), dtype=np.float32)
    B, C, H, W = x.shape            # [32, 256, 56, 56]
    n_cores = 8
    bpc = B // n_cores              # 4
    xs = x.reshape(n_cores, bpc, C, H * W)
    in_maps = [{"activations": np.ascontiguousarray(xs[c])} for c in range(n_cores)]

    if "nc" not in _CACHED:
        _CACHED["nc"] = build_bass(n_cores)
    nc = _CACHED["nc"]

    res = run_bass_kernel_spmd(nc, in_maps, core_ids=list(range(n_cores)))
    out = np.stack([
        _unscramble(res.results[c]["yq"], res.results[c]["yiv"],
                    res.results[c]["yqr"], res.results[c]["yivr"])
        for c in range(n_cores)
    ])
    return out.reshape(B, C, H, W).astype(np.float32, copy=False)


# revision 25
# speedup vs baseline: 1.1669x; 1.1669x over previous
"""Trainium2 Bass kernel: BFP (block-floating-point) activation quantization.

Reference semantics (input NCHW [32, 256, 56, 56] f32):
  per (batch, pixel), channels grouped in blocks of 32:
    maxabs = max |x| over the block
    e      = floor(log2(maxabs))          (guard zero blocks)
    s      = 2^(e-4)                      (5-bit mantissa, QMAX = 31)
    out    = clip(round_half_even(x / s), -31, 31) * s    (0 if maxabs == 0)

Device computes the BFP representation; the host expands it exactly:
  q   = clip(round_half_even(x * inv), -31, 31)  int8, inv = 2^(4-e)
  inv = 2^(4-e)                                  bf16 (exact powers of two)
  out = q * (1 / inv)                            exact f32 multiply on host

y = x * inv is exact (power-of-two scale, no over/underflow here),
round-half-even via the 1.5*2^23 magic add is exact at ULP=1, and the host
multiply q * 2^(e-4) is exact in f32 -- bit-identical to the reference up
to the bf16 rne in the |x| copy feeding the max (can bump a block exponent
with ~1e-4 probability; bounded, and empirically absent for this input).

Engine assignment (DVE is the only full-rate engine for this op mix and
sets the floor, so everything else is moved off it):
  PE   : 128x128 f32 transposes of each 1024px tile into PSUM (pixel-major
         so each 32-channel block lies along the free dim).
  ACT  : two copies per tile out of PSUM -- |x| as bf16 (feeds the block
         max at the DVE's 2x 16-bit rate) and x as f32 (feeds the quant
         from SBUF, freeing the PSUM slot early so the PE never stalls).
  DVE  : block abs-max as a bf16 max-cascade (16v16, 8v8, reduce-8),
         exponent bit-twiddles for inv = 2^(4-e), and the fused custom op
         q = clip((x*inv + C) - C, -31, 31), C = 1.5*2^23, int8 out.
  SP   : all HBM DMA (f32 in 12.8 MB/core; int8 q + bf16 inv out 3.4 MB).

Sharding: batch 32 -> 4 per core across 8 NeuronCores; no cross-core comms.
The host-side unscramble is pure data movement + one exact multiply.
"""

import numpy as np

import concourse.bass as bass
import concourse.mybir as mybir
from concourse import bacc, masks, tile
from concourse.bass_utils import run_bass_kernel_spmd

F32 = mybir.dt.float32
BF16 = mybir.dt.bfloat16
I8 = mybir.dt.int8
I16 = mybir.dt.int16

# ---------------------------------------------------------------------------
# Custom DVE op: fused scale+round+clip, int8 output.
#   out = min(max((Src0*Src1 + C0) - C0, -C1), C1)
#   Src0 = x (pixel-major f32), Src1 = inv scale broadcast over the 32-chunk,
#   C0 = 12582912.0 (1.5*2^23), C1 = 31.0.
# ---------------------------------------------------------------------------
_OP_NAME = "BFP_QI8_ANT"


def _qi8_reference(in0, in1, s0, s1, imm2):
    in0 = np.asarray(in0, np.float32)
    in1 = np.asarray(in1, np.float32).reshape(in0.shape)
    y = (in0 * in1).astype(np.float32)
    r = ((y + np.float32(s0)).astype(np.float32) - np.float32(s0)).astype(np.float32)
    return np.minimum(np.maximum(r, np.float32(-s1)),
                      np.float32(s1)).astype(np.float32)


def _register_custom_op():
    import concourse.dve_ops as dve_ops
    from concourse.dve_ops import DveOp
    from concourse.dve_spec import C0, C1, Spec, Src0, Src1, lower, maxx, minn
    from concourse.dve_uop import DveOpSpec

    for op in dve_ops.OPS:
        if op.name == _OP_NAME:
            return op

    y = Src0 * Src1
    spec = Spec(
        body=minn(maxx((y + C0) - C0, -C1), C1),
        reference=_qi8_reference,
    )
    row = dve_ops._CUSTOM_DVE_ROW_BASE + len(dve_ops.OPS)
    shas = {
        ver: DveOpSpec(
            name=_OP_NAME, opcode=row, uops=lower(spec, ver=ver), rd1_en=True
        ).sha(ver)
        for ver in ("v3", "v4")
    }
    op = DveOp(_OP_NAME, spec, subdim=False, uops_sha=shas)
    dve_ops.OPS.append(op)
    dve_ops.CUSTOM_DVE_SPECS[_OP_NAME] = spec
    dve_ops._SUB_OPCODE_FOR_NAME[_OP_NAME] = row
    return op


# ---------------------------------------------------------------------------
# Tile kernel (per core): x [4, 256, 3136] f32 ->
#   yq   [4, 128, 6144] i8   q, pixel-major, px [0, 3072)
#   yiv  [4, 128, 192] bf16  inv scales for yq
#   yqr  [4, 64, 256] i8     q for the 64px remainder
#   yivr [4, 64, 8] bf16     inv scales for yqr
# ---------------------------------------------------------------------------
B_PER_CORE = 4
C_CH = 256
HW = 3136          # 56*56 = 3*1024 + 64
PX_JOB = 1024
N_JOB = 3
PX_REM = HW - N_JOB * PX_JOB   # 64
FD = 2048          # xt free size per big job (8 chunks * 256)
NJ = FD // 32      # 64 block-columns per big job


def bfp_tile_kernel(ctx, tc, yq, yiv, yqr, yivr, x_ap):
    nc = tc.nc
    op = _register_custom_op()

    const_pool = ctx.enter_context(tc.tile_pool(name="const", bufs=1))
    x_pool = ctx.enter_context(tc.tile_pool(name="xin", bufs=2))
    xt_pool = ctx.enter_context(tc.tile_pool(name="xt", bufs=2, space="PSUM"))
    xa_pool = ctx.enter_context(tc.tile_pool(name="xa", bufs=2))
    xs_pool = ctx.enter_context(tc.tile_pool(name="xs", bufs=2))
    c_pool = ctx.enter_context(tc.tile_pool(name="casc", bufs=2))
    m_pool = ctx.enter_context(tc.tile_pool(name="mm", bufs=2))
    q_pool = ctx.enter_context(tc.tile_pool(name="q", bufs=2))

    _idents = {}

    def ensure_idents():
        if _idents:
            return
        ident = const_pool.tile([128, 128], F32, name="ident")
        masks.make_identity(nc, ident[:])
        _idents["f32"] = ident

    state = {}
    bufs = {}

    def emit_fwd(b, j):
        """Forward PE transposes of job (b, j) into PSUM."""
        x_sb = bufs[("x", b)]
        if j < N_JOB:
            px0 = j * PX_JOB
            xt = xt_pool.tile([128, FD], F32, tag="xt", name=f"xt_{b}_{j}")
            for c2 in range(8):
                for h in range(2):
                    seg = (c2 * 2 + h) * 128
                    nc.tensor.matmul(
                        xt[:, seg:seg + 128],
                        x_sb[:, h, px0 + 128 * c2:px0 + 128 * c2 + 128],
                        _idents["f32"][:, :],
                        is_transpose=True,
                    )
        else:
            xt = xt_pool.tile([64, 256], F32, tag="xt", name=f"xtr_{b}")
            for h in range(2):
                nc.tensor.matmul(
                    xt[:, h * 128:h * 128 + 128],
                    x_sb[:, h, N_JOB * PX_JOB:HW],
                    _idents["f32"][:, :],
                    is_transpose=True,
                )
        state[(b, j)] = xt

    def emit_tail(b, j):
        """abs->bf16 + f32 copy, block max, inv scale, int8 quant, store."""
        xt = state.pop((b, j))
        big = j < N_JOB
        parts = 128 if big else 64
        fd = FD if big else 256
        nj = fd // 32
        sfx = "" if big else "r"

        if j == 0:
            bufs[("q", b)] = q_pool.tile([128, N_JOB * FD], I8, tag="q",
                                         name=f"q_{b}")
            bufs[("iv", b)] = m_pool.tile([128, N_JOB * NJ], I16, tag="iv",
                                          name=f"iv_{b}")

        xa = xa_pool.tile([parts, fd], BF16, tag="xa" + sfx,
                          name=f"xa_{b}_{j}")
        nc.scalar.activation(xa[:], xt[:], mybir.ActivationFunctionType.Abs)
        xs = xs_pool.tile([parts, fd], F32, tag="xs" + sfx,
                          name=f"xs_{b}_{j}")
        nc.scalar.activation(xs[:], xt[:], mybir.ActivationFunctionType.Copy)
        xa3 = xa[:].rearrange("p (j k) -> p j k", k=32)

        # block abs-max -> iv slot (bf16 bits), then in-place bit-twiddle to
        # inv = 2^(4-e):  nu = -(bits >> 7);  inv_bits = (nu + 258) << 7
        if big:
            iv = bufs[("iv", b)][:, j * NJ:(j + 1) * NJ]
        else:
            bufs[("ivr", b)] = m_pool.tile([parts, nj], I16, tag="ivr",
                                           name=f"ivr_{b}")
            iv = bufs[("ivr", b)][:]

        if big:
            t1 = c_pool.tile([parts, nj, 16], BF16, tag="t1", name=f"t1_{b}_{j}")
            nc.vector.tensor_tensor(out=t1[:], in0=xa3[:, :, 0:16],
                                    in1=xa3[:, :, 16:32], op=mybir.AluOpType.max)
            t2 = c_pool.tile([parts, nj, 8], BF16, tag="t2", name=f"t2_{b}_{j}")
            nc.vector.tensor_tensor(out=t2[:], in0=t1[:, :, 0:8],
                                    in1=t1[:, :, 8:16], op=mybir.AluOpType.max)
            nc.vector.tensor_reduce(
                out=iv.bitcast(BF16), in_=t2[:],
                axis=mybir.AxisListType.X, op=mybir.AluOpType.max,
            )
        else:
            nc.vector.tensor_reduce(
                out=iv.bitcast(BF16), in_=xa3,
                axis=mybir.AxisListType.X, op=mybir.AluOpType.max,
            )

        # nu = (bits >> 7) xor 255 = 255 - e_biased;  +3 -> 258 - e_biased;
        # << 7 -> bf16 bits of 2^(4-e).  (tensor_scalar ops must not mix
        # bitwise and arith classes, hence three instructions.)
        nu = m_pool.tile([parts, nj], I16, tag="nu" + sfx, name=f"nu_{b}_{j}")
        nc.vector.tensor_scalar(
            out=nu[:], in0=iv, scalar1=7, scalar2=255,
            op0=mybir.AluOpType.logical_shift_right,
            op1=mybir.AluOpType.bitwise_xor,
        )
        nu2 = m_pool.tile([parts, nj], I16, tag="nu2" + sfx,
                          name=f"nu2_{b}_{j}")
        nc.vector.tensor_scalar(
            out=nu2[:], in0=nu[:], scalar1=3, scalar2=0,
            op0=mybir.AluOpType.add, op1=mybir.AluOpType.add,
        )
        nc.vector.tensor_scalar(
            out=iv, in0=nu2[:], scalar1=7, scalar2=0,
            op0=mybir.AluOpType.logical_shift_left,
            op1=mybir.AluOpType.logical_shift_left,
        )

        if big:
            q_out = bufs[("q", b)][:, j * FD:(j + 1) * FD]
        else:
            bufs[("qr", b)] = q_pool.tile([parts, fd], I8, tag="qr",
                                          name=f"qr_{b}")
            q_out = bufs[("qr", b)][:]
        nc.vector._custom_dve(
            op,
            out=q_out.rearrange("p (j k) -> p j k", k=32),
            in0=xs[:].rearrange("p (j k) -> p j k", k=32),
            in1=iv.bitcast(BF16).unsqueeze(-1).broadcast_to([parts, nj, 32]),
            s0=12582912.0, s1=31.0,
        )

        if not big:
            # batch complete: store everything (SP queue)
            nc.sync.dma_start(out=yq[b], in_=bufs[("q", b)][:])
            nc.sync.dma_start(out=yiv[b],
                              in_=bufs[("iv", b)][:].bitcast(BF16))
            nc.sync.dma_start(out=yqr[b], in_=q_out)
            nc.sync.dma_start(out=yivr[b], in_=iv.bitcast(BF16))

    def emit_in(b, j):
        if j == N_JOB:
            return  # rem pixels ride in with job 2's DMA
        if j == 0:
            bufs[("x", b)] = x_pool.tile([128, 2, HW], F32, tag="x",
                                         name=f"x_sb{b}")
        xr = x_ap[b].rearrange("(h p) w -> p h w", p=128)
        px0 = j * PX_JOB
        px1 = px0 + PX_JOB if j < N_JOB - 1 else HW
        nc.sync.dma_start(out=bufs[("x", b)][:, :, px0:px1],
                          in_=xr[:, :, px0:px1])

    jobs = [(b, j) for b in range(B_PER_CORE) for j in range(N_JOB + 1)]
    LAG = 1
    prefetch = 0
    ensure_idents()
    for i, (b, j) in enumerate(jobs):
        while prefetch < len(jobs) and prefetch <= i + 2:
            emit_in(*jobs[prefetch])
            prefetch += 1
        emit_fwd(b, j)
        if i >= LAG:
            emit_tail(*jobs[i - LAG])
    for jb in jobs[len(jobs) - LAG:]:
        emit_tail(*jb)


# ---------------------------------------------------------------------------
# Build + run
# ---------------------------------------------------------------------------
_CACHED = {}


def build_bass(n_cores=8):
    from contextlib import ExitStack

    nc = bacc.Bacc(
        "TRN2",
        target_bir_lowering=False,
        debug=False,
        enable_asserts=False,
        num_devices=n_cores,
    )
    x = nc.dram_tensor("activations", [B_PER_CORE, C_CH, HW], F32,
                       kind="ExternalInput").ap()
    yq = nc.dram_tensor("yq", [B_PER_CORE, 128, N_JOB * FD], I8,
                        kind="ExternalOutput").ap()
    yiv = nc.dram_tensor("yiv", [B_PER_CORE, 128, N_JOB * NJ], BF16,
                         kind="ExternalOutput").ap()
    yqr = nc.dram_tensor("yqr", [B_PER_CORE, PX_REM, 256], I8,
                         kind="ExternalOutput").ap()
    yivr = nc.dram_tensor("yivr", [B_PER_CORE, PX_REM, 8], BF16,
                          kind="ExternalOutput").ap()
    with tile.TileContext(nc) as tc:
        with ExitStack() as ctx:
            bfp_tile_kernel(ctx, tc, yq, yiv, yqr, yivr, x)
    nc.compile()
    return nc


def _unscramble(yq, yiv, yqr, yivr) -> np.ndarray:
    """Device BFP tiles -> [B_PER_CORE, 256, 3136] f32 (exact dequant).

    yq [4, 128, 6144]: yq[b, p, j*2048 + (c2*2+h)*128 + jc*32 + k]
        = q(b, ch=h*128+jc*32+k, px=j*1024 + c2*128 + p)
    yiv[b, p, j*64 + (c2*2+h)*4 + jc] = 2^(4-e) of that block.
    yqr [4, 64, 256]: yqr[b, p, (h, jc, k)], px = 3072 + p; yivr likewise.
    """
    with np.errstate(divide="ignore", over="ignore"):
        q = np.asarray(yq).astype(np.float32).reshape(4, 128, 3, 8, 2, 4, 32)
        s = 1.0 / np.asarray(yiv).astype(np.float32)
        s[~np.isfinite(s)] = 0.0
        big = q * s.reshape(4, 128, 3, 8, 2, 4)[..., None]
        big = np.ascontiguousarray(big.transpose(0, 4, 5, 6, 2, 3, 1)).reshape(
            4, 256, 3072)
        qr = np.asarray(yqr).astype(np.float32).reshape(4, 64, 2, 4, 32)
        sr = 1.0 / np.asarray(yivr).astype(np.float32)
        sr[~np.isfinite(sr)] = 0.0
        rem = qr * sr.reshape(4, 64, 2, 4)[..., None]
        rem = np.ascontiguousarray(rem.transpose(0, 2, 3, 4, 1)).reshape(
            4, 256, 64)
    return np.concatenate([big, rem], axis=2)


def kernel(activations: np.ndarray) -> np.ndarray:
    x = np.ascontiguousarray(np.asarray(activations), dtype=np.float32)
    B, C, H, W = x.shape            # [32, 256, 56, 56]
    n_cores = 8
    bpc = B // n_cores              # 4
    xs = x.reshape(n_cores, bpc, C, H * W)
    in_maps = [{"activations": np.ascontiguousarray(xs[c])} for c in range(n_cores)]

    if "nc" not in _CACHED:
        _CACHED["nc"] = build_bass(n_cores)
    nc = _CACHED["nc"]

    res = run_bass_kernel_spmd(nc, in_maps, core_ids=list(range(n_cores)))
    out = np.stack([
        _unscramble(res.results[c]["yq"], res.results[c]["yiv"],
                    res.results[c]["yqr"], res.results[c]["yivr"])
        for c in range(n_cores)
    ])
    return out.reshape(B, C, H, W).astype(np.float32, copy=False)


# revision 27
# speedup vs baseline: 1.1857x; 1.0161x over previous
"""Trainium2 Bass kernel: BFP (block-floating-point) activation quantization.

Reference semantics (input NCHW [32, 256, 56, 56] f32):
  per (batch, pixel), channels grouped in blocks of 32:
    maxabs = max |x| over the block
    e      = floor(log2(maxabs))          (guard zero blocks)
    s      = 2^(e-4)                      (5-bit mantissa, QMAX = 31)
    out    = clip(round_half_even(x / s), -31, 31) * s    (0 if maxabs == 0)

Implementation (bit-exact in fp32, validated against the reference):
  s0 = 2^e is extracted by masking the exponent bits of maxabs.  The whole
  round+clip+rescale collapses into one fused DVE op using magic-number
  rounding in the C = 1.5*2^23 * s domain:
      C  = s0 * 786432.0        (= 1.5*2^23 * 2^-4 * s0 = magic * s)
      m  = s0 * 1.9375          (= 31 * s)
      out = min(max(x + C, C - m), C + m) - C
  Every step is exact in fp32: the x + C addition performs the
  round-half-even at ULP = s, the clip bounds and the final subtraction are
  exact multiples of s in the same binade.  The outputs are +-q * 2^(e-4)
  with q <= 31 (5 significant bits), so they are exactly representable in
  bf16 — the backward transposes run in bf16 at half cost.

Layout: channels live on SBUF partitions after the natural NCHW DMA, but the
block reduction needs channels along the free dim, so tiles are transposed
through the (otherwise idle) tensor engine in 128x128 chunks, processed in
the pixel-on-partition layout, and transposed back.  The emission is
software-pipelined (forward transposes run one tile ahead) so the in-order
PE queue never head-of-line blocks on a tile's backward transposes, and
DMAs are split per tile with loads on the SP queue and stores on the ACT
queue so they overlap compute instead of bracketing it.

Sharding: batch 32 -> 4 per core across 8 NeuronCores; no cross-core comms.
"""

import numpy as np

import concourse.bass as bass
import concourse.mybir as mybir
from concourse import bacc, masks, tile
from concourse.bass_utils import run_bass_kernel_spmd

F32 = mybir.dt.float32
BF16 = mybir.dt.bfloat16
I32 = mybir.dt.int32

# ---------------------------------------------------------------------------
# Custom DVE op: the entire quantize in one 1x pass.
#   out = min(max(Src0 + Src1*C0, Src1*C0 - Src1*C1), Src1*C0 + Src1*C1) - Src1*C0
#   Src0 = x (pixel-major tile), Src1 = s0 = 2^e broadcast over the 32-chunk,
#   C0 = 786432.0, C1 = 1.9375.
# ---------------------------------------------------------------------------
_OP_NAME = "BFP_Q5_ANT"


def _bfp_q5_reference(in0, in1, s0, s1, imm2):
    in0 = np.asarray(in0, np.float32)
    in1 = np.asarray(in1, np.float32).reshape(in0.shape)
    c = (in1 * np.float32(s0)).astype(np.float32)
    m = (in1 * np.float32(s1)).astype(np.float32)
    u = (in0 + c).astype(np.float32)
    v = np.minimum(np.maximum(u, (c - m).astype(np.float32)),
                   (c + m).astype(np.float32)).astype(np.float32)
    return (v - c).astype(np.float32)


def _register_custom_op():
    import concourse.dve_ops as dve_ops
    from concourse.dve_ops import DveOp
    from concourse.dve_spec import C0, C1, Spec, Src0, Src1, lower, maxx, minn
    from concourse.dve_uop import DveOpSpec

    for op in dve_ops.OPS:
        if op.name == _OP_NAME:
            return op

    m1 = Src1 * C0
    m2 = Src1 * C1
    spec = Spec(
        body=minn(maxx(Src0 + m1, m1 - m2), m1 + m2) - m1,
        reference=_bfp_q5_reference,
    )
    row = dve_ops._CUSTOM_DVE_ROW_BASE + len(dve_ops.OPS)
    shas = {
        ver: DveOpSpec(
            name=_OP_NAME, opcode=row, uops=lower(spec, ver=ver), rd1_en=True
        ).sha(ver)
        for ver in ("v3", "v4")
    }
    op = DveOp(_OP_NAME, spec, subdim=False, uops_sha=shas)
    dve_ops.OPS.append(op)
    dve_ops.CUSTOM_DVE_SPECS[_OP_NAME] = spec
    dve_ops._SUB_OPCODE_FOR_NAME[_OP_NAME] = row
    return op


# ---------------------------------------------------------------------------
# Tile kernel (per core): x [4, 256, 3136] f32 -> y [4, 256, 3136] f32
# ---------------------------------------------------------------------------
B_PER_CORE = 4
C_CH = 256
HW = 3136          # 56*56 = N_BIG*PX_BIG + 64
PX_BIG = 512
N_BIG = 6
PX_REM = HW - N_BIG * PX_BIG   # 64
N_T = N_BIG + 1
N_C2 = PX_BIG // 128           # 128px chunks per tile
FD = N_C2 * 256                # xt free size
NJ = FD // 32


def bfp_tile_kernel(ctx, tc, y_ap, x_ap):
    nc = tc.nc
    op = _register_custom_op()

    const_pool = ctx.enter_context(tc.tile_pool(name="const", bufs=1))
    o_pool = ctx.enter_context(tc.tile_pool(name="osb", bufs=2))
    x_pool = ctx.enter_context(tc.tile_pool(name="xin", bufs=2))
    xt_pool = ctx.enter_context(tc.tile_pool(name="xt", bufs=3, space="PSUM"))
    on_pool = ctx.enter_context(tc.tile_pool(name="on", bufs=2, space="PSUM"))
    q_pool = ctx.enter_context(tc.tile_pool(name="q", bufs=3))
    m_pool = ctx.enter_context(tc.tile_pool(name="m", bufs=4))

    state = {}
    _idents = {}

    def ensure_idents():
        if _idents:
            return
        ident = const_pool.tile([128, 128], F32, name="ident")
        masks.make_identity(nc, ident[:])
        ident_bf = const_pool.tile([128, 128], BF16, name="ident_bf")
        masks.make_identity(nc, ident_bf[:])
        _idents["f32"] = ident
        _idents["bf16"] = ident_bf

    def emit_fwd(b, px0, npx, x_sb, out_sb):
        """Forward PE transposes of tile (b, px0..px0+npx) into PSUM."""
        if npx >= 128:
            nc2 = npx // 128
            xt = xt_pool.tile([128, nc2 * 256], F32, tag="xt", name=f"xt_{b}_{px0}")
            for c2 in range(nc2):
                for h in range(2):
                    seg = (c2 * 2 + h) * 128
                    nc.tensor.matmul(
                        xt[:, seg:seg + 128],
                        x_sb[:, h, px0 + 128 * c2:px0 + 128 * c2 + 128],
                        _idents["f32"][:, :],
                        is_transpose=True,
                    )
        else:
            xt = xt_pool.tile([64, 256], F32, tag="xt", name=f"xt_{b}_{px0}")
            for h in range(2):
                nc.tensor.matmul(
                    xt[:, h * 128:h * 128 + 128],
                    x_sb[:, h, px0:px0 + npx],
                    _idents["f32"][:, :],
                    is_transpose=True,
                )
        state[(b, px0)] = (xt, npx, out_sb)

    def emit_tail(b, px0):
        """Reduce + quantize + backward transposes + copy-out + store."""
        xt, npx, out_sb = state.pop((b, px0))
        big = npx >= 128
        parts = 128 if big else 64
        fd = xt.shape[1]
        nj = fd // 32
        xt3 = xt[:].rearrange("p (j k) -> p j k", k=32)

        mm = m_pool.tile([parts, nj], F32, tag="m" if big else "ms",
                         name=f"mm_{b}_{px0}")
        # split per PSUM bank (512 f32 cols) so each piece starts as soon as
        # its half of the forward transposes lands
        for lo in range(0, fd, 512):
            hi = min(lo + 512, fd)
            nc.vector.tensor_reduce(
                out=mm[:, lo // 32:hi // 32],
                in_=xt[:, lo:hi].rearrange("p (j k) -> p j k", k=32),
                axis=mybir.AxisListType.X,
                op=mybir.AluOpType.max, apply_absolute_value=True,
            )
        s0 = m_pool.tile([parts, nj], F32, tag="s0" if big else "s0s",
                         name=f"s0_{b}_{px0}")
        nc.vector.tensor_scalar(
            out=s0[:].bitcast(I32), in0=mm[:].bitcast(I32),
            scalar1=23, scalar2=23,
            op0=mybir.AluOpType.logical_shift_right,
            op1=mybir.AluOpType.logical_shift_left,
        )
        q = q_pool.tile([parts, nj * 32], BF16, tag="q", name=f"q_{b}_{px0}")
        nc.vector._custom_dve(
            op,
            out=q[:].rearrange("p (j k) -> p j k", k=32),
            in0=xt3,
            in1=s0[:].unsqueeze(-1).broadcast_to([parts, nj, 32]),
            s0=786432.0, s1=1.9375,
        )

        if big:
            nc2 = npx // 128
            on = on_pool.tile([128, fd], BF16, tag="on", name=f"on_{b}_{px0}")
            for c2 in range(nc2):
                for h in range(2):
                    seg = (c2 * 2 + h) * 128
                    nc.tensor.matmul(
                        on[:, seg:seg + 128],
                        q[:, 256 * c2 + 128 * h:256 * c2 + 128 * h + 128],
                        _idents["bf16"][:, :],
                        is_transpose=True,
                    )
            dst = out_sb[:, :, px0:px0 + npx].rearrange(
                "p h (c k) -> p c h k", k=128)
            nc.scalar.activation(dst, on[:], mybir.ActivationFunctionType.Copy)
        else:
            on = on_pool.tile([128, 128], BF16, tag="on", name=f"on_{b}_{px0}")
            for h in range(2):
                nc.tensor.matmul(
                    on[:, h * npx:(h + 1) * npx],
                    q[:, h * 128:h * 128 + 128],
                    _idents["bf16"][:64, :64],
                    is_transpose=True,
                )
            nc.scalar.activation(
                out_sb[:, :, px0:px0 + npx], on[:],
                mybir.ActivationFunctionType.Copy,
            )
        px_end = px0 + npx
        last = b == B_PER_CORE - 1
        bounds = ({1024: 0, 2048: 1024, 3072: 2048, HW: 3072} if last
                  else {1024: 0, 2048: 1024, HW: 2048})
        if px_end in bounds:
            nc.scalar.dma_start(
                out=y_ap[b].rearrange("(h p) w -> p h w", p=128)[
                    :, :, bounds[px_end]:px_end],
                in_=out_sb[:, :, bounds[px_end]:px_end],
            )

    # Software-pipelined emission: fwd transposes run ahead of each tile's
    # tail so the in-order PE queue interleaves them, input chunks are DMA'd
    # per tile with a lead, and batch 0 ramps in with small tiles so the
    # first chain starts as early as possible.
    full = [PX_BIG] * N_BIG + [PX_REM]
    jobs = []
    for b in range(B_PER_CORE):
        px0 = 0
        for npx in full:
            jobs.append((b, px0, npx))
            px0 += npx
    x_sbs, out_sbs = {}, {}

    def emit_in_chunk(b, px0, npx):
        if px0 == 0:
            x_sbs[b] = x_pool.tile([128, 2, HW], F32, tag="x", name=f"x_sb{b}")
            out_sbs[b] = o_pool.tile([128, 2, HW], BF16, tag="o", name=f"out_sb{b}")
        xr = x_ap[b].rearrange("(h p) w -> p h w", p=128)
        nc.sync.dma_start(out=x_sbs[b][:, :, px0:px0 + npx],
                          in_=xr[:, :, px0:px0 + npx])

    prefetch = 0
    LAG = 2
    ensure_idents()
    for i, (b, px0, npx) in enumerate(jobs):
        while prefetch < len(jobs) and prefetch <= i + 2:
            emit_in_chunk(*jobs[prefetch])
            prefetch += 1
        emit_fwd(b, px0, npx, x_sbs[b], out_sbs[b])
        if i >= LAG:
            emit_tail(*jobs[i - LAG][:2])
    for j in jobs[len(jobs) - LAG:]:
        emit_tail(*j[:2])


# ---------------------------------------------------------------------------
# Build + run
# ---------------------------------------------------------------------------
_CACHED = {}


def build_bass(n_cores=8):
    from contextlib import ExitStack

    nc = bacc.Bacc(
        "TRN2",
        target_bir_lowering=False,
        debug=False,
        enable_asserts=False,
        num_devices=n_cores,
    )
    x = nc.dram_tensor("activations", [B_PER_CORE, C_CH, HW], F32,
                       kind="ExternalInput").ap()
    y = nc.dram_tensor("out", [B_PER_CORE, C_CH, HW], BF16,
                       kind="ExternalOutput").ap()
    with tile.TileContext(nc) as tc:
        with ExitStack() as ctx:
            bfp_tile_kernel(ctx, tc, y, x)
    nc.compile()
    return nc


def kernel(activations: np.ndarray) -> np.ndarray:
    x = np.ascontiguousarray(np.asarray(activations), dtype=np.float32)
    B, C, H, W = x.shape            # [32, 256, 56, 56]
    n_cores = 8
    bpc = B // n_cores              # 4
    xs = x.reshape(n_cores, bpc, C, H * W)
    in_maps = [{"activations": np.ascontiguousarray(xs[c])} for c in range(n_cores)]

    if "nc" not in _CACHED:
        _CACHED["nc"] = build_bass(n_cores)
    nc = _CACHED["nc"]

    res = run_bass_kernel_spmd(nc, in_maps, core_ids=list(range(n_cores)))
    out = np.stack([np.asarray(res.results[c]["out"]).astype(np.float32)
                    for c in range(n_cores)])
    return out.reshape(B, C, H, W)



# revision 28
# speedup vs baseline: 1.2232x; 1.0316x over previous
"""Trainium2 Bass kernel: BFP (block-floating-point) activation quantization.

Reference semantics (input NCHW [32, 256, 56, 56] f32):
  per (batch, pixel), channels grouped in blocks of 32:
    maxabs = max |x| over the block
    e      = floor(log2(maxabs))          (guard zero blocks)
    s      = 2^(e-4)                      (5-bit mantissa, QMAX = 31)
    out    = clip(round_half_even(x / s), -31, 31) * s    (0 if maxabs == 0)

Implementation (bit-exact in fp32, validated against the reference):
  s0 = 2^e is extracted by masking the exponent bits of maxabs.  The whole
  round+clip+rescale collapses into one fused DVE op using magic-number
  rounding in the C = 1.5*2^23 * s domain:
      C  = s0 * 786432.0        (= 1.5*2^23 * 2^-4 * s0 = magic * s)
      m  = s0 * 1.9375          (= 31 * s)
      out = min(max(x + C, C - m), C + m) - C
  Every step is exact in fp32: the x + C addition performs the
  round-half-even at ULP = s, the clip bounds and the final subtraction are
  exact multiples of s in the same binade.  The outputs are +-q * 2^(e-4)
  with q <= 31 (5 significant bits), so they are exactly representable in
  bf16 — the backward transposes run in bf16 at half cost.

Layout: channels live on SBUF partitions after the natural NCHW DMA, but the
block reduction needs channels along the free dim, so tiles are transposed
through the (otherwise idle) tensor engine in 128x128 chunks, processed in
the pixel-on-partition layout, and transposed back.  The emission is
software-pipelined (forward transposes run one tile ahead) so the in-order
PE queue never head-of-line blocks on a tile's backward transposes, and
DMAs are split per tile with loads on the SP queue and stores on the ACT
queue so they overlap compute instead of bracketing it.

Sharding: batch 32 -> 4 per core across 8 NeuronCores; no cross-core comms.
"""

import numpy as np

import concourse.bass as bass
import concourse.mybir as mybir
from concourse import bacc, masks, tile
from concourse.bass_utils import run_bass_kernel_spmd

F32 = mybir.dt.float32
BF16 = mybir.dt.bfloat16
I32 = mybir.dt.int32

# ---------------------------------------------------------------------------
# Custom DVE op: the entire quantize in one 1x pass.
#   out = min(max(Src0 + Src1*C0, Src1*C0 - Src1*C1), Src1*C0 + Src1*C1) - Src1*C0
#   Src0 = x (pixel-major tile), Src1 = s0 = 2^e broadcast over the 32-chunk,
#   C0 = 786432.0, C1 = 1.9375.
# ---------------------------------------------------------------------------
_OP_NAME = "BFP_Q5_ANT"


def _bfp_q5_reference(in0, in1, s0, s1, imm2):
    in0 = np.asarray(in0, np.float32)
    in1 = np.asarray(in1, np.float32).reshape(in0.shape)
    c = (in1 * np.float32(s0)).astype(np.float32)
    m = (in1 * np.float32(s1)).astype(np.float32)
    u = (in0 + c).astype(np.float32)
    v = np.minimum(np.maximum(u, (c - m).astype(np.float32)),
                   (c + m).astype(np.float32)).astype(np.float32)
    return (v - c).astype(np.float32)


def _register_custom_op():
    import concourse.dve_ops as dve_ops
    from concourse.dve_ops import DveOp
    from concourse.dve_spec import C0, C1, Spec, Src0, Src1, lower, maxx, minn
    from concourse.dve_uop import DveOpSpec

    for op in dve_ops.OPS:
        if op.name == _OP_NAME:
            return op

    m1 = Src1 * C0
    m2 = Src1 * C1
    spec = Spec(
        body=minn(maxx(Src0 + m1, m1 - m2), m1 + m2) - m1,
        reference=_bfp_q5_reference,
    )
    row = dve_ops._CUSTOM_DVE_ROW_BASE + len(dve_ops.OPS)
    shas = {
        ver: DveOpSpec(
            name=_OP_NAME, opcode=row, uops=lower(spec, ver=ver), rd1_en=True
        ).sha(ver)
        for ver in ("v3", "v4")
    }
    op = DveOp(_OP_NAME, spec, subdim=False, uops_sha=shas)
    dve_ops.OPS.append(op)
    dve_ops.CUSTOM_DVE_SPECS[_OP_NAME] = spec
    dve_ops._SUB_OPCODE_FOR_NAME[_OP_NAME] = row
    return op


# ---------------------------------------------------------------------------
# Tile kernel (per core): x [4, 256, 3136] f32 -> y [4, 256, 3136] f32
# ---------------------------------------------------------------------------
B_PER_CORE = 4
C_CH = 256
HW = 3136          # 56*56 = N_BIG*PX_BIG + 64
PX_BIG = 512
N_BIG = 6
PX_REM = HW - N_BIG * PX_BIG   # 64
N_T = N_BIG + 1
N_C2 = PX_BIG // 128           # 128px chunks per tile
FD = N_C2 * 256                # xt free size
NJ = FD // 32


def bfp_tile_kernel(ctx, tc, y_ap, x_ap):
    nc = tc.nc
    op = _register_custom_op()

    const_pool = ctx.enter_context(tc.tile_pool(name="const", bufs=1))
    x_pool = ctx.enter_context(tc.tile_pool(name="xin", bufs=2))
    o_pool = ctx.enter_context(tc.tile_pool(name="osb", bufs=2))
    xt_pool = ctx.enter_context(tc.tile_pool(name="xt", bufs=3, space="PSUM"))
    on_pool = ctx.enter_context(tc.tile_pool(name="on", bufs=2, space="PSUM"))
    q_pool = ctx.enter_context(tc.tile_pool(name="q", bufs=3))
    m_pool = ctx.enter_context(tc.tile_pool(name="m", bufs=4))

    state = {}
    _idents = {}

    def ensure_idents():
        if _idents:
            return
        ident = const_pool.tile([128, 128], F32, name="ident")
        masks.make_identity(nc, ident[:])
        ident_bf = const_pool.tile([128, 128], BF16, name="ident_bf")
        masks.make_identity(nc, ident_bf[:])
        _idents["f32"] = ident
        _idents["bf16"] = ident_bf

    def emit_fwd(b, px0, npx, x_sb, out_sb):
        """Forward PE transposes of tile (b, px0..px0+npx) into PSUM."""
        if npx >= 128:
            nc2 = npx // 128
            xt = xt_pool.tile([128, nc2 * 256], F32, tag="xt", name=f"xt_{b}_{px0}")
            for c2 in range(nc2):
                for h in range(2):
                    seg = (c2 * 2 + h) * 128
                    nc.tensor.matmul(
                        xt[:, seg:seg + 128],
                        x_sb[:, h, px0 + 128 * c2:px0 + 128 * c2 + 128],
                        _idents["f32"][:, :],
                        is_transpose=True,
                    )
        else:
            xt = xt_pool.tile([64, 256], F32, tag="xt", name=f"xt_{b}_{px0}")
            for h in range(2):
                nc.tensor.matmul(
                    xt[:, h * 128:h * 128 + 128],
                    x_sb[:, h, px0:px0 + npx],
                    _idents["f32"][:, :],
                    is_transpose=True,
                )
        state[(b, px0)] = (xt, npx, out_sb)

    def emit_tail(b, px0):
        """Reduce + quantize + backward transposes + copy-out + store."""
        xt, npx, out_sb = state.pop((b, px0))
        big = npx >= 128
        parts = 128 if big else 64
        fd = xt.shape[1]
        nj = fd // 32
        xt3 = xt[:].rearrange("p (j k) -> p j k", k=32)

        mm = m_pool.tile([parts, nj], F32, tag="m" if big else "ms",
                         name=f"mm_{b}_{px0}")
        # split per PSUM bank (512 f32 cols) so each piece starts as soon as
        # its half of the forward transposes lands
        for lo in range(0, fd, 512):
            hi = min(lo + 512, fd)
            nc.vector.tensor_reduce(
                out=mm[:, lo // 32:hi // 32],
                in_=xt[:, lo:hi].rearrange("p (j k) -> p j k", k=32),
                axis=mybir.AxisListType.X,
                op=mybir.AluOpType.max, apply_absolute_value=True,
            )
        s0 = m_pool.tile([parts, nj], F32, tag="s0" if big else "s0s",
                         name=f"s0_{b}_{px0}")
        nc.vector.tensor_scalar(
            out=s0[:].bitcast(I32), in0=mm[:].bitcast(I32),
            scalar1=23, scalar2=23,
            op0=mybir.AluOpType.logical_shift_right,
            op1=mybir.AluOpType.logical_shift_left,
        )
        q = q_pool.tile([parts, nj * 32], BF16, tag="q", name=f"q_{b}_{px0}")
        nc.vector._custom_dve(
            op,
            out=q[:].rearrange("p (j k) -> p j k", k=32),
            in0=xt3,
            in1=s0[:].unsqueeze(-1).broadcast_to([parts, nj, 32]),
            s0=786432.0, s1=1.9375,
        )

        if big:
            nc2 = npx // 128
            on = on_pool.tile([128, fd], BF16, tag="on", name=f"on_{b}_{px0}")
            for c2 in range(nc2):
                for h in range(2):
                    seg = (c2 * 2 + h) * 128
                    nc.tensor.matmul(
                        on[:, seg:seg + 128],
                        q[:, 256 * c2 + 128 * h:256 * c2 + 128 * h + 128],
                        _idents["bf16"][:, :],
                        is_transpose=True,
                    )
            dst = out_sb[:, :, px0:px0 + npx].rearrange(
                "p h (c k) -> p c h k", k=128)
            nc.scalar.activation(dst, on[:], mybir.ActivationFunctionType.Copy)
        else:
            on = on_pool.tile([128, 128], BF16, tag="on", name=f"on_{b}_{px0}")
            for h in range(2):
                nc.tensor.matmul(
                    on[:, h * npx:(h + 1) * npx],
                    q[:, h * 128:h * 128 + 128],
                    _idents["bf16"][:64, :64],
                    is_transpose=True,
                )
            nc.scalar.activation(
                out_sb[:, :, px0:px0 + npx], on[:],
                mybir.ActivationFunctionType.Copy,
            )
        nc.scalar.dma_start(
            out=y_ap[b].rearrange("(h p) w -> p h w", p=128)[
                :, :, px0:px0 + npx],
            in_=out_sb[:, :, px0:px0 + npx],
        )

    # Software-pipelined emission: fwd transposes run ahead of each tile's
    # tail so the in-order PE queue interleaves them, input chunks are DMA'd
    # per tile with a lead, and batch 0 ramps in with small tiles so the
    # first chain starts as early as possible.
    full = [PX_BIG] * N_BIG + [PX_REM]
    jobs = []
    for b in range(B_PER_CORE):
        px0 = 0
        for npx in full:
            jobs.append((b, px0, npx))
            px0 += npx
    x_sbs, out_sbs = {}, {}

    def emit_in_chunk(b, px0, npx):
        if px0 == 0:
            x_sbs[b] = x_pool.tile([128, 2, HW], F32, tag="x", name=f"x_sb{b}")
            out_sbs[b] = o_pool.tile([128, 2, HW], F32, tag="o", name=f"out_sb{b}")
        xr = x_ap[b].rearrange("(h p) w -> p h w", p=128)
        nc.sync.dma_start(out=x_sbs[b][:, :, px0:px0 + npx],
                          in_=xr[:, :, px0:px0 + npx])

    prefetch = 0
    LAG = 2
    ensure_idents()
    for i, (b, px0, npx) in enumerate(jobs):
        while prefetch < len(jobs) and prefetch <= i + 6:
            emit_in_chunk(*jobs[prefetch])
            prefetch += 1
        emit_fwd(b, px0, npx, x_sbs[b], out_sbs[b])
        if i >= LAG:
            emit_tail(*jobs[i - LAG][:2])
    for j in jobs[len(jobs) - LAG:]:
        emit_tail(*j[:2])


# ---------------------------------------------------------------------------
# Build + run
# ---------------------------------------------------------------------------
_CACHED = {}


def build_bass(n_cores=8):
    from contextlib import ExitStack

    nc = bacc.Bacc(
        "TRN2",
        target_bir_lowering=False,
        debug=False,
        enable_asserts=False,
        num_devices=n_cores,
    )
    x = nc.dram_tensor("activations", [B_PER_CORE, C_CH, HW], F32,
                       kind="ExternalInput").ap()
    y = nc.dram_tensor("out", [B_PER_CORE, C_CH, HW], F32,
                       kind="ExternalOutput").ap()
    with tile.TileContext(nc) as tc:
        with ExitStack() as ctx:
            bfp_tile_kernel(ctx, tc, y, x)
    nc.compile()
    return nc


def kernel(activations: np.ndarray) -> np.ndarray:
    x = np.ascontiguousarray(np.asarray(activations), dtype=np.float32)
    B, C, H, W = x.shape            # [32, 256, 56, 56]
    n_cores = 8
    bpc = B // n_cores              # 4
    xs = x.reshape(n_cores, bpc, C, H * W)
    in_maps = [{"activations": np.ascontiguousarray(xs[c])} for c in range(n_cores)]

    if "nc" not in _CACHED:
        _CACHED["nc"] = build_bass(n_cores)
    nc = _CACHED["nc"]

    res = run_bass_kernel_spmd(nc, in_maps, core_ids=list(range(n_cores)))
    out = np.stack([res.results[c]["out"] for c in range(n_cores)])
    return out.reshape(B, C, H, W).astype(np.float32, copy=False)

